# revision 11
# baseline (speedup 1.0000x reference)
"""FP8 dynamic-quantized linear (x @ W + b with abs-max fp8 quantization).

Strategy (8 NeuronCores):
  - Shard: 2-way on flattened batch*seq rows of inp, 4-way column-wise on
    weight out_features.  Each core computes a [4096, 4096] block of the
    [8192, 16384] output (K = 4096 contraction on-device).
  - The two scalar quantization scales (global abs-max of inp / weight) are
    computed on host and replicated to every core as tiny input tensors.
  - Everything else (fp8 quantization of x and W, fp8 DoubleRow matmul,
    dequant scale + bias epilogue) runs on-device.

fp8 format note: TRN float8e4 (= ml_dtypes.float8_e4m3, max 240, has inf)
differs from the reference's OCP float8_e4m3fn (max 448).  We quantize with
half the reference scale so post-scale values live in [-224, 224]; on the
power-of-2-relative e4m3 grid the RNE rounding then matches the reference's
e4m3fn rounding exactly (up to a negligible subnormal tail), and the factor
of 4 (2x per operand) is folded into the fp32 dequant scale.
"""

import numpy as np

F8_MAX = np.float32(448.0)

# ---- problem geometry (hardcoded per the task spec) ----
B, T, K, OUTF = 4, 2048, 4096, 16384
ROWS = B * T                     # 8192
N_CORES = 8
ROW_SHARDS, COL_SHARDS = 1, 8
ROWS_C = ROWS // ROW_SHARDS      # 8192 rows per core (replicated x)
OUTF_C = OUTF // COL_SHARDS      # 2048 out-features per core

P = 128                          # SBUF partitions
KO = K // P                      # 32 k-subtiles
RT = ROWS_C // P                 # 64 row tiles per core
OC = 512                         # out-feature chunk (psum free dim)
NCHUNK = OUTF_C // OC            # 4 chunks per core
NPASS = 1                        # all 4 chunks resident, single pass
CPP = NCHUNK // NPASS            # chunks per pass = 4
KH = 8                           # ko-slices per w staging DMA


def _build_nc(rt=RT, ko=KO, nchunk=NCHUNK, oc=OC, warm_rt=12):
    """Build the per-core SPMD bass program (same program on all 8 cores).

    Structure: all `nchunk` fp8 weight chunks are SBUF-resident.  To hide the
    ~32 MiB cold-start weight load, the first `warm_rt` row tiles run a
    "warm-up" visit over chunks {0,1} only (ready after ~1/4 of the weight
    bytes); their quantized x tiles stay pinned in SBUF and the matching
    chunks {2,3} visits run as a tail mini-pass at full PE rate.
    """
    import concourse.bass as bass
    import concourse.tile as tile
    from concourse import bacc, mybir

    outf_c = nchunk * oc
    f32 = mybir.dt.float32
    f8 = mybir.dt.float8e4
    DR = mybir.MatmulPerfMode.DoubleRow
    warm_rt = min(warm_rt, rt)
    warm_chunks = nchunk // 2 if nchunk > 1 and warm_rt else 0

    nc = bacc.Bacc(
        "TRN2",
        target_bir_lowering=False,
        debug=False,
        enable_asserts=False,
        num_devices=N_CORES,
    )

    xt = nc.dram_tensor("xt", [rt, P, ko, P], f32, kind="ExternalInput").ap()
    wt = nc.dram_tensor("wt", [nchunk, P, ko, oc], f32, kind="ExternalInput").ap()
    biasb = nc.dram_tensor("biasb", [P, outf_c], f32, kind="ExternalInput").ap()
    consts = nc.dram_tensor("consts", [P, 4], f32, kind="ExternalInput").ap()
    out = nc.dram_tensor("out", [rt, P, outf_c], f32, kind="ExternalOutput").ap()

    kh = min(KH, ko)
    kho = ko // kh

    with tile.TileContext(nc) as tc:
        # DMA queue split: x loads ride the SP (sync) HWDGE FIFO; w loads and
        # out stores ride the ACT (scalar) HWDGE FIFO.  With a single FIFO the
        # next row's x load queues behind the previous row's out store (which
        # waits on its eviction), stalling the PE ~4.4us per row tile.
        with (
            tc.tile_pool(name="const", bufs=1) as const_pool,
            tc.tile_pool(name="wq", bufs=nchunk) as wq_pool,
            tc.tile_pool(name="xqw", bufs=max(warm_rt, 1)) as xqw_pool,
            tc.tile_pool(name="xq", bufs=3) as xq_pool,
            tc.tile_pool(name="xf", bufs=2) as xf_pool,
            tc.tile_pool(name="wf", bufs=2) as wf_pool,
            tc.tile_pool(name="osb", bufs=2) as out_pool,
            tc.tile_pool(name="psum", bufs=8, space="PSUM") as psum_pool,
        ):
            consts_t = const_pool.tile([P, 4], f32)
            nc.sync.dma_start(consts_t[:], consts)
            rx_half = consts_t[:, 0:1]
            rw_half = consts_t[:, 1:2]
            c4 = consts_t[:, 2:3]

            biasb_t = const_pool.tile([P, outf_c], f32)
            nc.scalar.dma_start(biasb_t[:], biasb)

            def load_wq_slice(wq_c, c, h):
                wf = wf_pool.tile([P, kh, oc], f32, tag="wf")
                nc.scalar.dma_start(wf[:], wt[c, :, h * kh:(h + 1) * kh, :])
                # w_q = fp8(w * (recip_w / 2)) on the scalar engine
                nc.scalar.mul(wq_c[:, h * kh:(h + 1) * kh, :], wf[:], rw_half)

            # kh-major interleaved loads within each chunk group: the first
            # k-slices of the group land first, so the PE k2 ladder can start
            # early; warm-up group {0,1} loads entirely before group {2,3}.
            wq_chunks = [wq_pool.tile([P, ko, oc], f8, tag="wq", name="wq")
                         for _ in range(nchunk)]
            groups = ([range(warm_chunks), range(warm_chunks, nchunk)]
                      if warm_chunks else [range(nchunk)])
            for grp in groups:
                for h in range(kho):
                    for c in grp:
                        load_wq_slice(wq_chunks[c], c, h)

            def load_xq(r, pool):
                xq = pool.tile([P, ko, P], f8, tag=f"xq{pool is xqw_pool}",
                               name="xq")
                for half in range(2):
                    ksl = slice(half * ko // 2, (half + 1) * ko // 2)
                    xf = xf_pool.tile([P, ko // 2, P], f32, tag="xf")
                    nc.sync.dma_start(xf[:], xt[r][:, ksl, :])
                    # x_q = fp8(x * (recip_x / 2)) on the vector engine
                    nc.vector.tensor_scalar_mul(xq[:, ksl, :], xf[:], rx_half)
                return xq

            def visit(r, xq, chunks, pos):
                psums = [
                    psum_pool.tile([P, oc], f32, space="PSUM",
                                   name="ps", tag="ps")
                    for _ in chunks
                ]
                for k2 in range(ko // 2):
                    lhsT = xq[:, 2 * k2:2 * k2 + 2, :]
                    for j, c in enumerate(chunks):
                        nc.tensor.matmul(
                            psums[j][:],
                            lhsT,
                            wq_chunks[c][:, 2 * k2:2 * k2 + 2, :],
                            start=(k2 == 0),
                            stop=(k2 == ko // 2 - 1),
                            perf_mode=DR,
                        )
                osb = out_pool.tile([P, len(chunks) * oc], f32, tag="osb",
                                    name="osb")
                for j, c in enumerate(chunks):
                    # out = psum * (4*sx*sw) + bias, fused on the vector engine
                    nc.vector.scalar_tensor_tensor(
                        osb[:, j * oc:(j + 1) * oc],
                        psums[j][:],
                        c4,
                        biasb_t[:, c * oc:(c + 1) * oc],
                        mybir.AluOpType.mult,
                        mybir.AluOpType.add,
                    )
                nc.scalar.dma_start(
                    out[r][:, pos * oc:(pos + len(chunks)) * oc], osb[:])

            if warm_chunks:
                warm_xqs = []
                # warm-up: first tiles against the early chunk group only
                for r in range(warm_rt):
                    xq = load_xq(r, xqw_pool)
                    warm_xqs.append(xq)
                    visit(r, xq, range(warm_chunks), 0)
                # main: all chunks
                for r in range(warm_rt, rt):
                    visit(r, load_xq(r, xq_pool), range(nchunk), 0)
                # tail mini-pass: warm tiles against the late chunk group
                for r in range(warm_rt):
                    visit(r, warm_xqs[r], range(warm_chunks, nchunk),
                          warm_chunks)
            else:
                for r in range(rt):
                    visit(r, load_xq(r, xq_pool), range(nchunk), 0)

    nc.compile()
    return nc


_NC_CACHE = {}


def _get_nc(key=None):
    if key not in _NC_CACHE:
        _NC_CACHE[key] = _build_nc()
    return _NC_CACHE[key]


def _host_scales(inp, weight):
    """Replicate the reference's fp32 scale arithmetic exactly."""
    amax_w = np.max(np.abs(weight)).astype(np.float32)
    w_scale = amax_w / F8_MAX
    recip_w = np.float32(1.0) / w_scale

    amax_x = np.max(np.abs(inp)).astype(np.float32)
    x_scale = amax_x / F8_MAX
    recip_x = np.float32(1.0) / x_scale

    c4 = np.float32(4.0) * (x_scale * w_scale)
    rx_half = recip_x * np.float32(0.5)
    rw_half = recip_w * np.float32(0.5)
    return rx_half, rw_half, c4


def kernel(inp, weight, bias):
    return _run(inp, weight, bias)[0]


def _run(inp, weight, bias, trace=False, **kwargs):
    from concourse.bass_utils import run_bass_kernel_spmd

    inp = np.asarray(inp)
    weight = np.asarray(weight)
    bias = np.asarray(bias)

    rx_half, rw_half, c4 = _host_scales(inp, weight)
    consts = np.zeros((P, 4), np.float32)
    consts[:, 0] = rx_half
    consts[:, 1] = rw_half
    consts[:, 2] = c4

    x2 = inp.reshape(ROWS, K)

    # Pre-tile x row-shards: xt[r, ki, ko, col] = x_shard[r*128+col, ko*128+ki]
    xts = []
    for s in range(ROW_SHARDS):
        xs = x2[s * ROWS_C:(s + 1) * ROWS_C]
        xt = np.ascontiguousarray(
            xs.reshape(RT, P, KO, P).transpose(0, 3, 2, 1))
        xts.append(xt)

    # Pre-tile w col-shards: wt[c, ki, ko, col] = w_shard[ko*128+ki, c*512+col]
    wts, biasbs = [], []
    for s in range(COL_SHARDS):
        ws = weight[:, s * OUTF_C:(s + 1) * OUTF_C]
        wt = np.ascontiguousarray(
            ws.reshape(KO, P, NCHUNK, OC).transpose(2, 1, 0, 3))
        wts.append(wt)
        bs = bias[s * OUTF_C:(s + 1) * OUTF_C]
        biasbs.append(np.ascontiguousarray(
            np.broadcast_to(bs[None, :], (P, OUTF_C))))

    in_maps = []
    for c in range(N_CORES):
        rs, cs = divmod(c, COL_SHARDS)
        in_maps.append({
            "xt": xts[rs],
            "wt": wts[cs],
            "biasb": biasbs[cs],
            "consts": consts,
        })

    nc = _get_nc()
    res = run_bass_kernel_spmd(
        nc, in_maps, core_ids=list(range(N_CORES)), trace=trace, **kwargs
    )

    full = np.empty((ROWS, OUTF), np.float32)
    for c in range(N_CORES):
        rs, cs = divmod(c, COL_SHARDS)
        blk = res.results[c]["out"].reshape(ROWS_C, OUTF_C)
        full[rs * ROWS_C:(rs + 1) * ROWS_C, cs * OUTF_C:(cs + 1) * OUTF_C] = blk
    return full.reshape(B, T, OUTF), res


# revision 15
# speedup vs baseline: 1.0429x; 1.0429x over previous
"""FP8 dynamic-quantized linear (x @ W + b with abs-max fp8 quantization).

Strategy (8 NeuronCores):
  - Shard: 2-way on flattened batch*seq rows of inp, 4-way column-wise on
    weight out_features.  Each core computes a [4096, 4096] block of the
    [8192, 16384] output (K = 4096 contraction on-device).
  - The two scalar quantization scales (global abs-max of inp / weight) are
    computed on host and replicated to every core as tiny input tensors.
  - Everything else (fp8 quantization of x and W, fp8 DoubleRow matmul,
    dequant scale + bias epilogue) runs on-device.

fp8 format note: TRN float8e4 (= ml_dtypes.float8_e4m3, max 240, has inf)
differs from the reference's OCP float8_e4m3fn (max 448).  We quantize with
half the reference scale so post-scale values live in [-224, 224]; on the
power-of-2-relative e4m3 grid the RNE rounding then matches the reference's
e4m3fn rounding exactly (up to a negligible subnormal tail), and the factor
of 4 (2x per operand) is folded into the fp32 dequant scale.
"""

import numpy as np

F8_MAX = np.float32(448.0)

# ---- problem geometry (hardcoded per the task spec) ----
B, T, K, OUTF = 4, 2048, 4096, 16384
ROWS = B * T                     # 8192
N_CORES = 8
ROW_SHARDS, COL_SHARDS = 1, 8
ROWS_C = ROWS // ROW_SHARDS      # 8192 rows per core (replicated x)
OUTF_C = OUTF // COL_SHARDS      # 2048 out-features per core

P = 128                          # SBUF partitions
KO = K // P                      # 32 k-subtiles
RT = ROWS_C // P                 # 64 row tiles per core
OC = 512                         # out-feature chunk (psum free dim)
NCHUNK = OUTF_C // OC            # 4 chunks per core
NPASS = 1                        # all 4 chunks resident, single pass
CPP = NCHUNK // NPASS            # chunks per pass = 4
KH = 8                           # ko-slices per w staging DMA


def _build_nc(rt=RT, ko=KO, nchunk=NCHUNK, oc=OC, warm_rt=8):
    """Build the per-core SPMD bass program (same program on all 8 cores).

    Structure: all `nchunk` fp8 weight chunks are SBUF-resident.  To hide the
    ~32 MiB cold-start weight load, the first `warm_rt` row tiles run a
    "warm-up" visit over chunks {0,1} only (ready after ~1/4 of the weight
    bytes); their quantized x tiles stay pinned in SBUF and the matching
    chunks {2,3} visits run as a tail mini-pass at full PE rate.
    """
    import concourse.bass as bass
    import concourse.tile as tile
    from concourse import bacc, mybir

    outf_c = nchunk * oc
    f32 = mybir.dt.float32
    f8 = mybir.dt.float8e4
    DR = mybir.MatmulPerfMode.DoubleRow
    warm_rt = min(warm_rt, rt)
    warm_chunks = nchunk // 2 if nchunk > 1 and warm_rt else 0

    nc = bacc.Bacc(
        "TRN2",
        target_bir_lowering=False,
        debug=False,
        enable_asserts=False,
        num_devices=N_CORES,
    )

    xt = nc.dram_tensor("xt", [rt, P, ko, P], f32, kind="ExternalInput").ap()
    wt = nc.dram_tensor("wt", [nchunk, P, ko, oc], f32, kind="ExternalInput").ap()
    biasb = nc.dram_tensor("biasb", [P, outf_c], f32, kind="ExternalInput").ap()
    consts = nc.dram_tensor("consts", [P, 4], f32, kind="ExternalInput").ap()
    out = nc.dram_tensor("out", [rt, P, outf_c], f32, kind="ExternalOutput").ap()

    kh = min(KH, ko)
    kho = ko // kh

    with tile.TileContext(nc) as tc:
        # DMA queue split: x loads ride the SP (sync) HWDGE FIFO; w loads and
        # out stores ride the ACT (scalar) HWDGE FIFO.  With a single FIFO the
        # next row's x load queues behind the previous row's out store (which
        # waits on its eviction), stalling the PE ~4.4us per row tile.
        with (
            tc.tile_pool(name="const", bufs=1) as const_pool,
            tc.tile_pool(name="wq", bufs=nchunk) as wq_pool,
            tc.tile_pool(name="xqw", bufs=max(warm_rt, 1)) as xqw_pool,
            tc.tile_pool(name="xq", bufs=3) as xq_pool,
            tc.tile_pool(name="xf", bufs=2) as xf_pool,
            tc.tile_pool(name="wf", bufs=3) as wf_pool,
            tc.tile_pool(name="osb", bufs=2) as out_pool,
            tc.tile_pool(name="psum", bufs=8, space="PSUM") as psum_pool,
        ):
            consts_t = const_pool.tile([P, 4], f32)
            nc.sync.dma_start(consts_t[:], consts)
            rx_half = consts_t[:, 0:1]
            rw_half = consts_t[:, 1:2]
            c4 = consts_t[:, 2:3]

            biasb_t = const_pool.tile([P, outf_c], f32)
            nc.scalar.dma_start(biasb_t[:], biasb)

            def load_wq_slice(wq_c, c, h, eng):
                wf = wf_pool.tile([P, kh, oc], f32, tag="wf")
                nc.scalar.dma_start(wf[:], wt[c, :, h * kh:(h + 1) * kh, :])
                # w_q = fp8(w * (recip_w / 2)); alternate ACT/DVE so the
                # quantize doesn't serialize the w DMA stream on one engine
                if eng:
                    nc.scalar.mul(wq_c[:, h * kh:(h + 1) * kh, :], wf[:], rw_half)
                else:
                    nc.vector.tensor_scalar_mul(
                        wq_c[:, h * kh:(h + 1) * kh, :], wf[:], rw_half)

            # kh-major interleaved loads within each chunk group: the first
            # k-slices of the group land first, so the PE k2 ladder can start
            # early; warm-up group {0,1} loads entirely before group {2,3}.
            wq_chunks = [wq_pool.tile([P, ko, oc], f8, tag="wq", name="wq")
                         for _ in range(nchunk)]
            groups = ([range(warm_chunks), range(warm_chunks, nchunk)]
                      if warm_chunks else [range(nchunk)])
            eng = 0
            for grp in groups:
                for h in range(kho):
                    for c in grp:
                        load_wq_slice(wq_chunks[c], c, h, eng)
                        eng ^= 1

            def load_xq(r, pool):
                xq = pool.tile([P, ko, P], f8, tag=f"xq{pool is xqw_pool}",
                               name="xq")
                for half in range(2):
                    ksl = slice(half * ko // 2, (half + 1) * ko // 2)
                    xf = xf_pool.tile([P, ko // 2, P], f32, tag="xf")
                    nc.sync.dma_start(xf[:], xt[r][:, ksl, :])
                    # x_q = fp8(x * (recip_x / 2)) on the vector engine
                    nc.vector.tensor_scalar_mul(xq[:, ksl, :], xf[:], rx_half)
                return xq

            def visit(r, xq, chunks, pos):
                psums = [
                    psum_pool.tile([P, oc], f32, space="PSUM",
                                   name="ps", tag="ps")
                    for _ in chunks
                ]
                for k2 in range(ko // 2):
                    lhsT = xq[:, 2 * k2:2 * k2 + 2, :]
                    for j, c in enumerate(chunks):
                        nc.tensor.matmul(
                            psums[j][:],
                            lhsT,
                            wq_chunks[c][:, 2 * k2:2 * k2 + 2, :],
                            start=(k2 == 0),
                            stop=(k2 == ko // 2 - 1),
                            perf_mode=DR,
                        )
                osb = out_pool.tile([P, len(chunks) * oc], f32, tag="osb",
                                    name="osb")
                for j, c in enumerate(chunks):
                    # out = psum * (4*sx*sw) + bias, fused on the vector engine
                    nc.vector.scalar_tensor_tensor(
                        osb[:, j * oc:(j + 1) * oc],
                        psums[j][:],
                        c4,
                        biasb_t[:, c * oc:(c + 1) * oc],
                        mybir.AluOpType.mult,
                        mybir.AluOpType.add,
                    )
                nc.scalar.dma_start(
                    out[r][:, pos * oc:(pos + len(chunks)) * oc], osb[:])

            if warm_chunks:
                warm_xqs = []
                # warm-up: first tiles against the early chunk group only
                for r in range(warm_rt):
                    xq = load_xq(r, xqw_pool)
                    warm_xqs.append(xq)
                    visit(r, xq, range(warm_chunks), 0)
                # main: all chunks
                for r in range(warm_rt, rt):
                    visit(r, load_xq(r, xq_pool), range(nchunk), 0)
                # tail mini-pass: warm tiles against the late chunk group
                for r in range(warm_rt):
                    visit(r, warm_xqs[r], range(warm_chunks, nchunk),
                          warm_chunks)
            else:
                for r in range(rt):
                    visit(r, load_xq(r, xq_pool), range(nchunk), 0)

    nc.compile()
    return nc


_NC_CACHE = {}


def _get_nc(key=None):
    if key not in _NC_CACHE:
        _NC_CACHE[key] = _build_nc()
    return _NC_CACHE[key]


def _host_scales(inp, weight):
    """Replicate the reference's fp32 scale arithmetic exactly."""
    amax_w = np.max(np.abs(weight)).astype(np.float32)
    w_scale = amax_w / F8_MAX
    recip_w = np.float32(1.0) / w_scale

    amax_x = np.max(np.abs(inp)).astype(np.float32)
    x_scale = amax_x / F8_MAX
    recip_x = np.float32(1.0) / x_scale

    c4 = np.float32(4.0) * (x_scale * w_scale)
    rx_half = recip_x * np.float32(0.5)
    rw_half = recip_w * np.float32(0.5)
    return rx_half, rw_half, c4


def kernel(inp, weight, bias):
    return _run(inp, weight, bias)[0]


def _run(inp, weight, bias, trace=False, **kwargs):
    from concourse.bass_utils import run_bass_kernel_spmd

    inp = np.asarray(inp)
    weight = np.asarray(weight)
    bias = np.asarray(bias)

    rx_half, rw_half, c4 = _host_scales(inp, weight)
    consts = np.zeros((P, 4), np.float32)
    consts[:, 0] = rx_half
    consts[:, 1] = rw_half
    consts[:, 2] = c4

    x2 = inp.reshape(ROWS, K)

    # Pre-tile x row-shards: xt[r, ki, ko, col] = x_shard[r*128+col, ko*128+ki]
    xts = []
    for s in range(ROW_SHARDS):
        xs = x2[s * ROWS_C:(s + 1) * ROWS_C]
        xt = np.ascontiguousarray(
            xs.reshape(RT, P, KO, P).transpose(0, 3, 2, 1))
        xts.append(xt)

    # Pre-tile w col-shards: wt[c, ki, ko, col] = w_shard[ko*128+ki, c*512+col]
    wts, biasbs = [], []
    for s in range(COL_SHARDS):
        ws = weight[:, s * OUTF_C:(s + 1) * OUTF_C]
        wt = np.ascontiguousarray(
            ws.reshape(KO, P, NCHUNK, OC).transpose(2, 1, 0, 3))
        wts.append(wt)
        bs = bias[s * OUTF_C:(s + 1) * OUTF_C]
        biasbs.append(np.ascontiguousarray(
            np.broadcast_to(bs[None, :], (P, OUTF_C))))

    in_maps = []
    for c in range(N_CORES):
        rs, cs = divmod(c, COL_SHARDS)
        in_maps.append({
            "xt": xts[rs],
            "wt": wts[cs],
            "biasb": biasbs[cs],
            "consts": consts,
        })

    nc = _get_nc()
    res = run_bass_kernel_spmd(
        nc, in_maps, core_ids=list(range(N_CORES)), trace=trace, **kwargs
    )

    full = np.empty((ROWS, OUTF), np.float32)
    for c in range(N_CORES):
        rs, cs = divmod(c, COL_SHARDS)
        blk = res.results[c]["out"].reshape(ROWS_C, OUTF_C)
        full[rs * ROWS_C:(rs + 1) * ROWS_C, cs * OUTF_C:(cs + 1) * OUTF_C] = blk
    return full.reshape(B, T, OUTF), res


# revision 17
# speedup vs baseline: 1.0825x; 1.0379x over previous
"""FP8 dynamic-quantized linear (x @ W + b with abs-max fp8 quantization).

Strategy (8 NeuronCores):
  - Shard: 2-way on flattened batch*seq rows of inp, 4-way column-wise on
    weight out_features.  Each core computes a [4096, 4096] block of the
    [8192, 16384] output (K = 4096 contraction on-device).
  - The two scalar quantization scales (global abs-max of inp / weight) are
    computed on host and replicated to every core as tiny input tensors.
  - Everything else (fp8 quantization of x and W, fp8 DoubleRow matmul,
    dequant scale + bias epilogue) runs on-device.

fp8 format note: TRN float8e4 (= ml_dtypes.float8_e4m3, max 240, has inf)
differs from the reference's OCP float8_e4m3fn (max 448).  We quantize with
half the reference scale so post-scale values live in [-224, 224]; on the
power-of-2-relative e4m3 grid the RNE rounding then matches the reference's
e4m3fn rounding exactly (up to a negligible subnormal tail), and the factor
of 4 (2x per operand) is folded into the fp32 dequant scale.
"""

import numpy as np

F8_MAX = np.float32(448.0)

# ---- problem geometry (hardcoded per the task spec) ----
B, T, K, OUTF = 4, 2048, 4096, 16384
ROWS = B * T                     # 8192
N_CORES = 8
ROW_SHARDS, COL_SHARDS = 1, 8
ROWS_C = ROWS // ROW_SHARDS      # 8192 rows per core (replicated x)
OUTF_C = OUTF // COL_SHARDS      # 2048 out-features per core

P = 128                          # SBUF partitions
KO = K // P                      # 32 k-subtiles
RT = ROWS_C // P                 # 64 row tiles per core
OC = 512                         # out-feature chunk (psum free dim)
NCHUNK = OUTF_C // OC            # 4 chunks per core, all SBUF-resident as fp8
KH = 8                           # ko-slices per w staging DMA


def _build_nc(rt=RT, ko=KO, nchunk=NCHUNK, oc=OC, warm_rt=0):
    """Build the per-core SPMD bass program (same program on all 8 cores).

    Structure: all `nchunk` fp8 weight chunks are SBUF-resident.  To hide the
    ~32 MiB cold-start weight load, the first `warm_rt` row tiles run a
    "warm-up" visit over chunks {0,1} only (ready after ~1/4 of the weight
    bytes); their quantized x tiles stay pinned in SBUF and the matching
    chunks {2,3} visits run as a tail mini-pass at full PE rate.
    """
    import concourse.bass as bass
    import concourse.tile as tile
    from concourse import bacc, mybir

    outf_c = nchunk * oc
    f32 = mybir.dt.float32
    f8 = mybir.dt.float8e4
    DR = mybir.MatmulPerfMode.DoubleRow
    warm_rt = min(warm_rt, rt)
    warm_chunks = nchunk // 2 if nchunk > 1 and warm_rt else 0

    nc = bacc.Bacc(
        "TRN2",
        target_bir_lowering=False,
        debug=False,
        enable_asserts=False,
        num_devices=N_CORES,
    )

    xt = nc.dram_tensor("xt", [rt, P, ko, P], f32, kind="ExternalInput").ap()
    wt = nc.dram_tensor("wt", [nchunk, P, ko, oc], f32, kind="ExternalInput").ap()
    biasb = nc.dram_tensor("biasb", [P, outf_c], f32, kind="ExternalInput").ap()
    consts = nc.dram_tensor("consts", [P, 4], f32, kind="ExternalInput").ap()
    out = nc.dram_tensor("out", [rt, P, outf_c], f32, kind="ExternalOutput").ap()

    kh = min(KH, ko)
    kho = ko // kh

    with tile.TileContext(nc) as tc:
        # DMA queue split: x loads ride the SP (sync) HWDGE FIFO; w loads and
        # out stores ride the ACT (scalar) HWDGE FIFO.  With a single FIFO the
        # next row's x load queues behind the previous row's out store (which
        # waits on its eviction), stalling the PE ~4.4us per row tile.
        with (
            tc.tile_pool(name="const", bufs=1) as const_pool,
            tc.tile_pool(name="wq", bufs=nchunk) as wq_pool,
            tc.tile_pool(name="xqw", bufs=max(warm_rt, 1)) as xqw_pool,
            tc.tile_pool(name="xq", bufs=3) as xq_pool,
            tc.tile_pool(name="xf", bufs=2) as xf_pool,
            tc.tile_pool(name="wf", bufs=3) as wf_pool,
            tc.tile_pool(name="osb", bufs=2) as out_pool,
            tc.tile_pool(name="psum", bufs=8, space="PSUM") as psum_pool,
        ):
            consts_t = const_pool.tile([P, 4], f32)
            nc.sync.dma_start(consts_t[:], consts)
            rx_half = consts_t[:, 0:1]
            rw_half = consts_t[:, 1:2]
            c4 = consts_t[:, 2:3]

            biasb_t = const_pool.tile([P, outf_c], f32)
            nc.scalar.dma_start(biasb_t[:], biasb)

            def load_wq_slice(wq_c, c, h, eng):
                wf = wf_pool.tile([P, kh, oc], f32, tag="wf")
                nc.scalar.dma_start(wf[:], wt[c, :, h * kh:(h + 1) * kh, :])
                # w_q = fp8(w * (recip_w / 2)); alternate ACT/DVE so the
                # quantize doesn't serialize the w DMA stream on one engine
                if eng:
                    nc.scalar.mul(wq_c[:, h * kh:(h + 1) * kh, :], wf[:], rw_half)
                else:
                    nc.vector.tensor_scalar_mul(
                        wq_c[:, h * kh:(h + 1) * kh, :], wf[:], rw_half)

            # kh-major interleaved loads within each chunk group: the first
            # k-slices of the group land first, so the PE k2 ladder can start
            # early; warm-up group {0,1} loads entirely before group {2,3}.
            wq_chunks = [wq_pool.tile([P, ko, oc], f8, tag="wq", name="wq")
                         for _ in range(nchunk)]
            groups = ([range(warm_chunks), range(warm_chunks, nchunk)]
                      if warm_chunks else [range(nchunk)])
            eng = 0
            for grp in groups:
                for h in range(kho):
                    for c in grp:
                        load_wq_slice(wq_chunks[c], c, h, eng)
                        eng ^= 1

            def load_xq(r, pool):
                xq = pool.tile([P, ko, P], f8, tag=f"xq{pool is xqw_pool}",
                               name="xq")
                for half in range(2):
                    ksl = slice(half * ko // 2, (half + 1) * ko // 2)
                    xf = xf_pool.tile([P, ko // 2, P], f32, tag="xf")
                    nc.sync.dma_start(xf[:], xt[r][:, ksl, :])
                    # x_q = fp8(x * (recip_x / 2)) on the vector engine
                    nc.vector.tensor_scalar_mul(xq[:, ksl, :], xf[:], rx_half)
                return xq

            def visit(r, xq, chunks, pos):
                psums = [
                    psum_pool.tile([P, oc], f32, space="PSUM",
                                   name="ps", tag="ps")
                    for _ in chunks
                ]
                for k2 in range(ko // 2):
                    lhsT = xq[:, 2 * k2:2 * k2 + 2, :]
                    for j, c in enumerate(chunks):
                        nc.tensor.matmul(
                            psums[j][:],
                            lhsT,
                            wq_chunks[c][:, 2 * k2:2 * k2 + 2, :],
                            start=(k2 == 0),
                            stop=(k2 == ko // 2 - 1),
                            perf_mode=DR,
                        )
                osb = out_pool.tile([P, len(chunks) * oc], f32, tag="osb",
                                    name="osb")
                for j, c in enumerate(chunks):
                    # out = psum * (4*sx*sw) + bias, fused on the vector engine
                    nc.vector.scalar_tensor_tensor(
                        osb[:, j * oc:(j + 1) * oc],
                        psums[j][:],
                        c4,
                        biasb_t[:, c * oc:(c + 1) * oc],
                        mybir.AluOpType.mult,
                        mybir.AluOpType.add,
                    )
                nc.scalar.dma_start(
                    out[r][:, pos * oc:(pos + len(chunks)) * oc], osb[:])

            if warm_chunks:
                warm_xqs = []
                # warm-up: first tiles against the early chunk group only
                for r in range(warm_rt):
                    xq = load_xq(r, xqw_pool)
                    warm_xqs.append(xq)
                    visit(r, xq, range(warm_chunks), 0)
                # main: all chunks
                for r in range(warm_rt, rt):
                    visit(r, load_xq(r, xq_pool), range(nchunk), 0)
                # tail mini-pass: warm tiles against the late chunk group
                for r in range(warm_rt):
                    visit(r, warm_xqs[r], range(warm_chunks, nchunk),
                          warm_chunks)
            else:
                for r in range(rt):
                    visit(r, load_xq(r, xq_pool), range(nchunk), 0)

    nc.compile()
    return nc


_NC_CACHE = {}


def _get_nc(key=None):
    if key not in _NC_CACHE:
        _NC_CACHE[key] = _build_nc()
    return _NC_CACHE[key]


def _host_scales(inp, weight):
    """Replicate the reference's fp32 scale arithmetic exactly."""
    amax_w = np.max(np.abs(weight)).astype(np.float32)
    w_scale = amax_w / F8_MAX
    recip_w = np.float32(1.0) / w_scale

    amax_x = np.max(np.abs(inp)).astype(np.float32)
    x_scale = amax_x / F8_MAX
    recip_x = np.float32(1.0) / x_scale

    c4 = np.float32(4.0) * (x_scale * w_scale)
    rx_half = recip_x * np.float32(0.5)
    rw_half = recip_w * np.float32(0.5)
    return rx_half, rw_half, c4


def kernel(inp, weight, bias):
    return _run(inp, weight, bias)[0]


def _run(inp, weight, bias, trace=False, **kwargs):
    from concourse.bass_utils import run_bass_kernel_spmd

    inp = np.asarray(inp)
    weight = np.asarray(weight)
    bias = np.asarray(bias)

    rx_half, rw_half, c4 = _host_scales(inp, weight)
    consts = np.zeros((P, 4), np.float32)
    consts[:, 0] = rx_half
    consts[:, 1] = rw_half
    consts[:, 2] = c4

    x2 = inp.reshape(ROWS, K)

    # Pre-tile x row-shards: xt[r, ki, ko, col] = x_shard[r*128+col, ko*128+ki]
    xts = []
    for s in range(ROW_SHARDS):
        xs = x2[s * ROWS_C:(s + 1) * ROWS_C]
        xt = np.ascontiguousarray(
            xs.reshape(RT, P, KO, P).transpose(0, 3, 2, 1))
        xts.append(xt)

    # Pre-tile w col-shards: wt[c, ki, ko, col] = w_shard[ko*128+ki, c*512+col]
    wts, biasbs = [], []
    for s in range(COL_SHARDS):
        ws = weight[:, s * OUTF_C:(s + 1) * OUTF_C]
        wt = np.ascontiguousarray(
            ws.reshape(KO, P, NCHUNK, OC).transpose(2, 1, 0, 3))
        wts.append(wt)
        bs = bias[s * OUTF_C:(s + 1) * OUTF_C]
        biasbs.append(np.ascontiguousarray(
            np.broadcast_to(bs[None, :], (P, OUTF_C))))

    in_maps = []
    for c in range(N_CORES):
        rs, cs = divmod(c, COL_SHARDS)
        in_maps.append({
            "xt": xts[rs],
            "wt": wts[cs],
            "biasb": biasbs[cs],
            "consts": consts,
        })

    nc = _get_nc()
    res = run_bass_kernel_spmd(
        nc, in_maps, core_ids=list(range(N_CORES)), trace=trace, **kwargs
    )

    full = np.empty((ROWS, OUTF), np.float32)
    for c in range(N_CORES):
        rs, cs = divmod(c, COL_SHARDS)
        blk = res.results[c]["out"].reshape(ROWS_C, OUTF_C)
        full[rs * ROWS_C:(rs + 1) * ROWS_C, cs * OUTF_C:(cs + 1) * OUTF_C] = blk
    return full.reshape(B, T, OUTF), res
